# revision 28
# baseline (speedup 1.0000x reference)
"""Causal GQA self-attention (RoPE + QK-RMSNorm) Trainium2 kernel.

Sharding: 8 cores = batch (2) x kv-head-group (4). Each core computes, for
its (batch b, kv-group g): the 4 query heads + 1 kv head of that group,
causal attention over the full sequence, and a partial output projection
y_bg = O_g @ W_O[rows of group g]. Host sums the 4 partials per batch.

Device layout is "transposed" throughout: activations live as [feature,
token] so every matmul contracts over the partition axis with 512-wide
moving operands. Matmuls run in bf16 (f32 PSUM accumulation); softmax,
RoPE and RMS statistics stay f32.

v2 changes vs baseline (349.9us):
 - exp for two 128-row score tiles is batched into ONE activation over a
   2-bank PSUM tile (halves ACT instruction overhead; ACT was ~100% busy
   through the attention phase).
 - causal masking via memset-zero + 128-wide triangle multiply on the
   exp'd tile (drops the full-width praw copy/mask DVE traffic).
 - softmax denominator: DVE running sum of the exp'd tiles + a single
   ones-matmul per head-stream + DVE reciprocal (removes ~24 PE row
   matmuls and 32 ACT row ops).
 - yout tiles woven between attention score/AV pairs so the PE always
   has independent matmul work while ACT streams exps.
 - y stored bf16, one DMA per 128-token stripe; x loaded with one
   rearrange DMA per later block (fewer sync-queue DMA instructions).
"""

import numpy as np
import ml_dtypes

import concourse.bass as bass  # noqa: F401
import concourse.tile as tile
from concourse import bacc, mybir
from concourse import bass_utils

BF16 = mybir.dt.bfloat16
F32 = mybir.dt.float32
NPBF16 = ml_dtypes.bfloat16

P = 128          # partitions == head_dim
HALF = 64        # rope half-dim
TB = 512         # t-block (psum free width)
S = 128          # s-tile (score partition block)
EPS = float(np.finfo(np.float32).eps)


class _one_act_table:
    """Steer Bacc's activation-table chooser to the single set that holds
    every function this kernel uses (Copy/Identity/Square/Ln/Exp), so the
    ScalarE never thrashes ACT_TABLE_LOADs between Ln and Exp. Set order
    (and therefore act_func_set_id indices) is preserved; original tables
    are restored on exit."""

    KEEP = "natural_log_exp_and_others"
    FUNCS = None  # filled lazily

    def __enter__(self):
        import concourse.hw_specs as hw
        import concourse.bacc as bacc_mod
        A = mybir.ActivationFunctionType
        if _one_act_table.FUNCS is None:
            _one_act_table.FUNCS = {A.Copy, A.Identity, A.Square, A.Ln,
                                    A.Exp, A.MemsetZero}
        self._orig = hw.get_activation_tables

        def patched(arch):
            tabs = self._orig(arch)
            return {k: (set(s) if k == self.KEEP else set(s) - self.FUNCS)
                    for k, s in tabs.items()}

        hw.get_activation_tables = patched
        bacc_mod.get_activation_tables = patched
        return self

    def __exit__(self, *exc):
        import concourse.hw_specs as hw
        import concourse.bacc as bacc_mod
        hw.get_activation_tables = self._orig
        bacc_mod.get_activation_tables = self._orig
        return False


def _build(T, C, G, n_devices=8):
    """Build the single-core SPMD program. T seq len, C model dim, G q-heads."""
    NB = T // TB         # t-blocks
    NC = C // P          # contraction tiles for projections
    SPB = TB // S        # s-tiles per t-block (4)
    NS = T // S          # s-tiles total
    DQ = G * P
    TPB = TB // P        # t-tiles (128 rows) per block
    NYB = C // TB        # y column blocks

    nc = bacc.Bacc("TRN2", target_bir_lowering=False, debug=False,
                   num_devices=n_devices)

    xT = nc.dram_tensor("xT", [C, T], BF16, kind="ExternalInput").ap()
    wq = nc.dram_tensor("wq", [C, DQ], BF16, kind="ExternalInput").ap()
    wk = nc.dram_tensor("wk", [C, P], BF16, kind="ExternalInput").ap()
    wv = nc.dram_tensor("wv", [C, P], BF16, kind="ExternalInput").ap()
    wo = nc.dram_tensor("wo", [DQ, C], BF16, kind="ExternalInput").ap()
    ccd = nc.dram_tensor("cc", [P, T], F32, kind="ExternalInput").ap()
    ssd = nc.dram_tensor("ss", [P, T], F32, kind="ExternalInput").ap()
    y = nc.dram_tensor("y", [T, C], BF16, kind="ExternalOutput").ap()

    # strictly-lower-triangle-killing mask for the 128-wide diagonal tile:
    # valid iff p <= f
    pp = np.arange(P)[:, None]
    ff = np.arange(S)[None, :]
    tri_np = np.where(pp <= ff, 1.0, 0.0).astype(NPBF16)
    tri_d = nc.inline_tensor(tri_np, "tri").ap()
    idn_d = nc.inline_tensor(np.eye(P, dtype=NPBF16), "idn").ap()
    onesb_d = nc.inline_tensor(np.ones((P, 1), NPBF16), "onesb").ap()

    with tile.TileContext(nc) as tc:
        with (
            tc.tile_pool(name="const", bufs=1) as const,
            tc.tile_pool(name="resid", bufs=1) as resid,
            tc.tile_pool(name="xp", bufs=2) as xp,
            tc.tile_pool(name="work", bufs=3) as work,
            tc.tile_pool(name="rows", bufs=3) as rows,
            tc.tile_pool(name="pp", bufs=3) as ppool,
            tc.tile_pool(name="yp", bufs=2) as yp,
            tc.tile_pool(name="ps_mm", bufs=2, space="PSUM") as ps_mm,
            tc.tile_pool(name="ps_o", bufs=2, space="PSUM") as ps_o,
            tc.tile_pool(name="ps_r", bufs=2, space="PSUM") as ps_r,
        ):
            # ---- constants into SBUF, ordered by first use: wv first (one
            # small DMA), then x block 0 in geometrically growing chunk
            # groups so the first projection matmuls start as soon as the
            # first chunk lands; wk goes on the scalar ring in parallel. ----
            wv_sb = const.tile([P, NC, P], BF16, tag="wv")
            nc.sync.dma_start(wv_sb[:, 0, :], wv[0:P, :])
            xs0 = xp.tile([P, NC, TB], BF16, tag="xs", name="xs0")
            nc.sync.dma_start(xs0[:, 0, :], xT[0:P, 0:TB])
            for ci in range(1, NC):
                nc.sync.dma_start(wv_sb[:, ci, :], wv[ci * P:(ci + 1) * P, :])
            for lo, hi in ((1, 2), (2, 4), (4, 8), (8, NC)):
                nc.sync.dma_start(
                    xs0[:, lo:hi, :],
                    xT[lo * P:hi * P, 0:TB].rearrange(
                        "(ci p) t -> p ci t", p=P))
            wk_sb = const.tile([P, NC, P], BF16, tag="wk")
            nc.scalar.dma_start(wk_sb, wk.rearrange("(ci p) j -> p ci j", p=P))
            idn = const.tile([P, P], BF16, tag="idn")
            nc.scalar.dma_start(idn, idn_d)
            ones_b = const.tile([P, 1], BF16, tag="onesb")
            nc.scalar.dma_start(ones_b, onesb_d)
            tri = const.tile([P, S], BF16, tag="tri")
            nc.scalar.dma_start(tri, tri_d)
            wq_sb = const.tile([P, NC, DQ], BF16, tag="wq")
            for h in range(G):
                nc.sync.dma_start(
                    wq_sb[:, :, h * P:(h + 1) * P],
                    wq[:, h * P:(h + 1) * P].rearrange(
                        "(ci p) j -> p ci j", p=P))
            cc_sb = const.tile([P, 2, TB], F32, tag="cc")
            nc.sync.dma_start(cc_sb[:, 0, :], ccd[:, 0:TB])
            ss_sb = const.tile([P, 2, TB], F32, tag="ss")
            nc.sync.dma_start(ss_sb[:, 0, :], ssd[:, 0:TB])
            wo_sb = const.tile([P, G, C], BF16, tag="wo")
            eps_q = const.tile([P, 1], F32, tag="epsq")
            nc.vector.memset(eps_q, P * EPS)
            eps_k = const.tile([P, 1], F32, tag="epsk")
            nc.vector.memset(eps_k, EPS)

            # ---- resident per-block activations (fine-grained for deps) ----
            qT = [[resid.tile([P, TB], BF16, tag=f"qT{h}_{j}",
                              name=f"qT{h}_{j}") for j in range(NB)]
                  for h in range(G)]
            kT = [resid.tile([P, TB], BF16, tag=f"kT{j}", name=f"kT{j}")
                  for j in range(NB)]
            vN = [resid.tile([P, P], BF16, tag=f"v{si}", name=f"v{si}")
                  for si in range(NS)]
            oT = [[resid.tile([P, TB], BF16, tag=f"oT{h}_{j}",
                              name=f"oT{h}_{j}") for j in range(NB)]
                  for h in range(G)]

            # rope/rms is software-pipelined in TWO deferred stages so neither
            # the DVE queue nor the PE ever block on a cross-engine chain:
            #  rope_pre: the two cos/sin multiplies + the half-swap DMA.
            #  flush_a (>=1 matmul group later): qr = a + rot (the DVE add
            #    would otherwise head-block the DVE queue on the DMA) and the
            #    ACT square.
            #  flush_b (later still): stat ones-matmul + Ln/Exp + broadcast +
            #    the final normalize into the resident qT/kT tile.
            pend_a = []
            pend_b = []
            flushed = set()

            def rope_pre(ps, dest, j, is_q):
                # u = q * [-sin; sin]; rotate_half(u) == rotate_half(q)*[sin; -sin]
                u = work.tile([P, TB], F32, tag="rm", bufs=3)
                nc.vector.tensor_mul(u, ps, ss_sb[:, j % 2, :])
                a = work.tile([P, TB], F32, tag="ra", bufs=3)
                nc.vector.tensor_mul(a, ps, cc_sb[:, j % 2, :])
                rot = work.tile([P, TB], F32, tag="rot", bufs=3)
                nc.gpsimd.dma_start(rot[0:HALF, :], u[HALF:P, :])
                nc.gpsimd.dma_start(rot[HALF:P, :], u[0:HALF, :])
                pend_a.append((a, rot, dest, is_q))

            def flush_a():
                if not pend_a:
                    return
                a, rot, dest, is_q = pend_a.pop(0)
                qr = work.tile([P, TB], F32, tag="qr", bufs=4)
                nc.vector.tensor_add(qr, a, rot)
                q2 = work.tile([P, TB], BF16, tag="q2", bufs=4, name="q2")
                nc.scalar.activation(q2, qr, mybir.ActivationFunctionType.Square)
                pend_b.append((qr, q2, dest, is_q))

            def flush_b():
                if not pend_b:
                    return
                qr, q2, dest, is_q = pend_b.pop(0)
                srow = ps_r.tile([1, TB], F32, tag="row", name="srow")
                nc.tensor.matmul(srow, ones_b, q2, start=True, stop=True)
                # inv = (scale*sum + eps')^-0.5 computed as exp(-0.5*ln(.))
                sq = rows.tile([1, TB], F32, tag="sq")
                if is_q:   # 1/sqrt(sum+d*eps) == rsqrt(mean+eps)/sqrt(d)
                    nc.scalar.activation(sq, srow,
                                         mybir.ActivationFunctionType.Ln,
                                         bias=eps_q[:1, :], scale=1.0)
                else:
                    nc.scalar.activation(sq, srow,
                                         mybir.ActivationFunctionType.Ln,
                                         bias=eps_k[:1, :], scale=1.0 / P)
                inv = rows.tile([1, TB], F32, tag="inv")
                nc.scalar.activation(inv, sq,
                                     mybir.ActivationFunctionType.Exp,
                                     scale=-0.5)
                invb = work.tile([P, TB], F32, tag="invb", bufs=2)
                nc.gpsimd.partition_broadcast(invb, inv)
                nc.vector.tensor_mul(dest, qr, invb)
                flushed.add(id(dest))

            def ensure_flushed(dest):
                while id(dest) not in flushed:
                    if pend_b:
                        flush_b()
                    elif pend_a:
                        flush_a()
                    else:
                        raise AssertionError("rope dest never enqueued")

            xs_pref = {}

            def emit_prefetch(jn):
                """Issue next block's x / cos / sin loads a full block early
                so the sync ring's 2MB transfer never gates anything."""
                blk = slice(jn * TB, (jn + 1) * TB)
                nc.sync.dma_start(cc_sb[:, jn % 2, :], ccd[:, blk])
                nc.sync.dma_start(ss_sb[:, jn % 2, :], ssd[:, blk])
                xs = xp.tile([P, NC, TB], BF16, tag="xs")
                nc.sync.dma_start(
                    xs, xT[:, blk].rearrange("(ci p) t -> p ci t", p=P))
                xs_pref[jn] = xs

            def emit_proj_vk(j):
                """V+K projections interleaved per contraction chunk into one
                2-bank PSUM tile (so block 0 compute chases the x DMA at 2
                matmuls per chunk)."""
                xs = xs0 if j == 0 else xs_pref.pop(j)
                ps = ps_mm.tile([P, 2, TB], F32, tag="mm", name="psvk")
                for ci in range(NC):
                    nc.tensor.matmul(ps[:, 0, :], wv_sb[:, ci, :],
                                     xs[:, ci, :],
                                     start=(ci == 0), stop=(ci == NC - 1))
                    nc.tensor.matmul(ps[:, 1, :], wk_sb[:, ci, :],
                                     xs[:, ci, :],
                                     start=(ci == 0), stop=(ci == NC - 1))
                rope_pre(ps[:, 1, :], kT[j], j, False)
                vp = work.tile([P, TB], BF16, tag="vp", bufs=2)
                nc.vector.tensor_copy(vp, ps[:, 0, :])
                return xs, vp

            def emit_transposes(j, vp):
                """V transposes, deferred so they never head-block the PE
                queue while the vp copy waits its turn on the DVE."""
                for k4 in range(SPB):
                    pt = ps_mm.tile([P, P], BF16, tag="mm", name="pt")
                    nc.tensor.transpose(pt, vp[:, k4 * P:(k4 + 1) * P], idn)
                    nc.vector.tensor_copy(vN[j * SPB + k4], pt)

            def emit_proj_q(j, h, xs):
                ps = ps_mm.tile([P, TB], F32, tag="mm", name="psq")
                for ci in range(NC):
                    nc.tensor.matmul(ps, wq_sb[:, ci, h * P:(h + 1) * P],
                                     xs[:, ci, :],
                                     start=(ci == 0), stop=(ci == NC - 1))
                flush_b()
                flush_a()
                rope_pre(ps, qT[h][j], j, True)

            def emit_yout_tile(jb, ti, yb, ys):
                """One [128,512] tile of the output projection for token-row
                ti; DMAs the full 128-token stripe once its last column block
                is done. The last block's copies go to ACT (idle at the
                tail) instead of the DVE."""
                yps = ps_mm.tile([P, TB], F32, tag="mm", name="yps")
                for h in range(G):
                    nc.tensor.matmul(
                        yps,
                        oT[h][jb][:, (ti % TPB) * P:(ti % TPB + 1) * P],
                        wo_sb[:, h, yb * TB:(yb + 1) * TB],
                        start=(h == 0), stop=(h == G - 1))
                nc.vector.tensor_copy(ys[:, yb, :], yps)
                if yb == NYB - 1:
                    nc.sync.dma_start(
                        y[ti * P:(ti + 1) * P, :].rearrange(
                            "p (yb t) -> p yb t", yb=NYB), ys)

            def emit_attn_head(j, h, fillers=()):
                """Causal attention stream (scores -> exp -> AV) for head h of
                t-block j; `fillers` are zero-arg emit thunks of independent
                PE work woven between pairs to cover ACT latency."""
                flush_b()
                flush_a()
                ensure_flushed(kT[j])
                ensure_flushed(qT[h][j])
                fillers = list(fillers)
                ns = (j + 1) * SPB
                npair = ns // 2
                oac = ps_o.tile([P, TB], F32, tag="oac")
                rrow = ps_r.tile([1, TB], F32, tag="row", name="rrow")
                pexs = [None] * npair

                def emit_scores(a):
                    """Scores pair a: 2 matmuls into a 2-bank PSUM tile,
                    batched exp, causal masking on the exp'd tile. On the
                    diagonal, columns left of the first valid token are
                    skipped in the matmul/exp and memset to zero instead."""
                    si0, si1 = 2 * a, 2 * a + 1
                    sps = ps_mm.tile([P, 2, TB], F32, tag="mm", name="sps")
                    ws = []
                    for i, si in ((0, si0), (1, si1)):
                        o = si - j * SPB
                        w = S * o if o > 0 else 0
                        ws.append(w)
                        nc.tensor.matmul(
                            sps[:, i, w:TB],
                            kT[si // SPB][:, (si % SPB) * S:(si % SPB + 1) * S],
                            qT[h][j][:, w:TB], start=True, stop=True)
                    pex = ppool.tile([P, 2, TB], BF16, tag="p", bufs=3,
                                     name="pex")
                    if ws[0] == ws[1]:
                        nc.scalar.activation(pex[:, :, ws[0]:TB],
                                             sps[:, :, ws[0]:TB],
                                             mybir.ActivationFunctionType.Exp)
                    else:
                        for i in (0, 1):
                            nc.scalar.activation(pex[:, i, ws[i]:TB],
                                                 sps[:, i, ws[i]:TB],
                                                 mybir.ActivationFunctionType.Exp)
                    for i, si in ((0, si0), (1, si1)):
                        o = si - j * SPB
                        if o >= 1:
                            nc.vector.memset(pex[:, i, 0:S * o], 0.0)
                        if o >= 0:
                            nc.vector.tensor_mul(pex[:, i, S * o:S * (o + 1)],
                                                 pex[:, i, S * o:S * (o + 1)],
                                                 tri)
                    pexs[a] = (pex, ws)

                # scores run one pair ahead of AV so the PE never waits on
                # the exp of the pair it is about to consume.
                emit_scores(0)
                if fillers:
                    fillers.pop(0)()
                for a in range(npair):
                    if a + 1 < npair:
                        emit_scores(a + 1)
                    pex, ws = pexs[a]
                    pexs[a] = None
                    nc.tensor.matmul(oac[:, ws[0]:TB], vN[2 * a],
                                     pex[:, 0, ws[0]:TB],
                                     start=(a == 0), stop=False)
                    nc.tensor.matmul(oac[:, ws[1]:TB], vN[2 * a + 1],
                                     pex[:, 1, ws[1]:TB],
                                     start=False, stop=(a == npair - 1))
                    psum2 = ppool.tile([P, TB], BF16, tag="p2", bufs=2,
                                       name="psum2")
                    # the first pair-sum of a stream goes to the (idle)
                    # gpsimd so the rowsum matmul isn't gated by whatever
                    # is still queued on the DVE from the previous stream
                    if a == 0:
                        nc.gpsimd.tensor_add(psum2, pex[:, 0, :],
                                             pex[:, 1, :])
                    else:
                        nc.vector.tensor_add(psum2, pex[:, 0, :],
                                             pex[:, 1, :])
                    nc.tensor.matmul(rrow, ones_b, psum2,
                                     start=(a == 0), stop=(a == npair - 1))
                    if fillers:
                        fillers.pop(0)()
                rln = rows.tile([1, TB], F32, tag="rln")
                nc.scalar.activation(rln, rrow,
                                     mybir.ActivationFunctionType.Ln)
                rinv = rows.tile([1, TB], F32, tag="rinv")
                nc.scalar.activation(rinv, rln,
                                     mybir.ActivationFunctionType.Exp,
                                     scale=-1.0)
                rb = work.tile([P, TB], F32, tag="rb", bufs=2)
                nc.gpsimd.partition_broadcast(rb, rinv)
                nc.vector.tensor_mul(oT[h][j], oac, rb)
                for f in fillers:
                    f()

            def yout_fillers(jb, h, ys_box):
                """Thunks for the 4 column blocks of token-row jb*TPB+h."""
                ti = jb * TPB + h

                def mk(yb):
                    def f():
                        if yb == 0:
                            ys_box[0] = yp.tile([P, NYB, TB], BF16, tag="ys",
                                                name="ys")
                        emit_yout_tile(jb, ti, yb, ys_box[0])
                    return f
                return [mk(yb) for yb in range(NYB)]

            # interleave: projections of block j run alongside attention of
            # block j-1 and the output projection of block j-2; yout tiles
            # are woven between attention pairs so the PE stays busy while
            # ACT streams the exps. Each attention stream is emitted BEFORE
            # the same head's Q projection so the DVE queue (masks/pair-sums
            # the PE needs now) is never stuck behind rope multiplies that
            # wait on projection matmuls.
            for j in range(NB):
                xs, vp = emit_proj_vk(j)
                for h in range(G):
                    if j >= 1:
                        fillers = []
                        if j >= 2:
                            fillers = yout_fillers(j - 2, h, [None])
                        emit_attn_head(j - 1, h, fillers)
                    if h == 2:
                        emit_transposes(j, vp)
                    emit_proj_q(j, h, xs)
                if j == 0:
                    nc.sync.dma_start(
                        wo_sb, wo.rearrange("(g p) c -> p g c", p=P))
                if j + 1 < NB:
                    emit_prefetch(j + 1)
            # tail: last block's attention woven with block NB-2's output
            # projection, then the final block's output projection.
            for h in range(G):
                emit_attn_head(NB - 1, h,
                               yout_fillers(NB - 2, h, [None])
                               if NB >= 2 else ())
            for h in range(G):
                ys_box = [None]
                for f in yout_fillers(NB - 1, h, ys_box):
                    f()

    with _one_act_table():
        nc.compile()
    return nc


_NC_CACHE = {}


def _get_nc(T, C, G):
    key = (T, C, G)
    if key not in _NC_CACHE:
        _NC_CACHE[key] = _build(T, C, G)
    return _NC_CACHE[key]


def _host_prep(x, cos, sin, W_Q, W_K, W_V, W_O, G):
    """Build the 8 per-core input maps (batch-major, then kv-group)."""
    B, T, C = x.shape
    n_kv = W_K.shape[1] // P
    cosT = np.ascontiguousarray(cos.reshape(T, HALF).T.astype(np.float32))
    sinT = np.ascontiguousarray(sin.reshape(T, HALF).T.astype(np.float32))
    cc = np.concatenate([cosT, cosT], axis=0)            # [128, T]
    ss = np.concatenate([-sinT, sinT], axis=0)           # [128, T]
    in_maps = []
    for b in range(B):
        xTb = np.ascontiguousarray(x[b].T).astype(NPBF16)
        for g in range(n_kv):
            in_maps.append({
                "xT": xTb,
                "wq": np.ascontiguousarray(
                    W_Q[:, g * G * P:(g + 1) * G * P]).astype(NPBF16),
                "wk": np.ascontiguousarray(
                    W_K[:, g * P:(g + 1) * P]).astype(NPBF16),
                "wv": np.ascontiguousarray(
                    W_V[:, g * P:(g + 1) * P]).astype(NPBF16),
                "wo": np.ascontiguousarray(
                    W_O[g * G * P:(g + 1) * G * P, :]).astype(NPBF16),
                "cc": cc,
                "ss": ss,
            })
    return in_maps


def kernel(x, cos, sin, W_Q, W_K, W_V, W_O):
    B, T, C = x.shape
    n_kv = W_K.shape[1] // P
    n_head = W_Q.shape[1] // P
    G = n_head // n_kv
    x = np.asarray(x, dtype=np.float32)
    nc = _get_nc(T, C, G)
    in_maps = _host_prep(x, np.asarray(cos), np.asarray(sin),
                         np.asarray(W_Q), np.asarray(W_K), np.asarray(W_V),
                         np.asarray(W_O), G)
    res = bass_utils.run_bass_kernel_spmd(
        nc, in_maps, core_ids=list(range(B * n_kv)))
    out = np.zeros((B, T, C), dtype=np.float32)
    for b in range(B):
        for g in range(n_kv):
            out[b] += np.asarray(res.results[b * n_kv + g]["y"],
                                 dtype=np.float32)
    return out
